# revision 13
# baseline (speedup 1.0000x reference)
"""Trainium2 Bass kernel for CorrCosine cost volumes.

Inputs (full): ref_features, cur_features [8, 256, 64, 64] f32.
out[b, hc, wc, hr, wr] = <cur_n[b, :, hc, wc], ref_n[b, :, hr, wr]>
where *_n are channel-L2-normalized features.

Sharding: data-parallel over batch B=8 across the 8 NeuronCores — each core
computes one batch's full [4096, 4096] cosine-similarity matrix:
  sim = (cur / |cur|_C).T @ (ref / |ref|_C)   with K = C = 256.

Per-core structure (Tile framework), all-bf16 pipeline (measured rates from
For_i microbenchmarks on this HW: PE 512 matmuls 113 us, 36 MB DMA 108 us
(~333 GB/s), 128 paired PSUM->SBUF copies 77 us):
  - host converts both inputs to bf16 (4 MB in); output written bf16
    (32 MB out, upcast to f32 on the host). bf16 matmuls run 1 cycle/row
    at 2.4 GHz unconditionally.
  - fill phase: ref slices stream on the HWDGE/sync queue, cur slices on
    the SWDGE/gpsimd queue in parallel. BOTH tensors are fully normalized
    in place during the fill: square (DVE 2x-packed bf16 / ACT alternating)
    -> all-ones [128,128] bf16 matmul per K-chunk (partition-reduce AND
    broadcast in one 1-cycle/row op) -> ACT Rsqrt (f32 PSUM -> bf16) ->
    two in-place bf16 muls (DVE 2x mode). No per-row inverse-norm folding
    remains in the main loop, and no N=1 column-reduce matmuls exist at
    all (they cost PE ~15 us in LDWEIGHTS/overhead in earlier revisions).
  - main loop: per m-chunk 4 PSUM pair-tiles [128, 1024] (2 banks each,
    psmm bufs=4 = all 8 banks so PE never waits on copy drain), 4 bf16
    matmuls per pair, pure paired copies (f32 PSUM -> bf16 SBUF)
    alternating ScalarE/VectorE, 3 MB output DMAs alternating the
    sync/gpsimd queues (measured best).
  - norm-phase PSUM is carved from halves of psmm pair tiles so the main
    loop owns all 8 banks once the fill drains.
  - rel-err ~4e-3 overall vs the 2e-2 gate.

loop_iters>1 wraps the body in a tc.For_i hardware loop for timing: device
exec then spans hundreds of ms, which is measurable through the axon tunnel
(single-shot exec hides entirely under the ~90 ms dispatch latency).
"""

import numpy as np

import concourse.bass as bass
import concourse.mybir as mybir
import concourse.tile as tile
from concourse import bacc, bass_utils

B, C, H, W = 8, 256, 64, 64
HW = H * W           # 4096 pixels
KP = 128             # partitions per K-chunk
NK = C // KP         # 2 K-chunks
MT = 128             # output partition tile (cur pixels)
NT = 512             # output free tile (ref pixels) = one f32 PSUM bank
NM = HW // MT        # 32 m-chunks
NN = HW // NT        # 8 n-tiles
NP = NN // 2         # 4 n-pairs per m-chunk
MO = 3               # m-chunks per output DMA (3 MB bf16 per dma_start)

F32 = mybir.dt.float32
BF16 = mybir.dt.bfloat16
ACTF = mybir.ActivationFunctionType


def _kernel_body(tc, cur, ref, sim, loop_iters=1):
    nc = tc.nc
    with (
        tc.tile_pool(name="pers", bufs=1) as pers,
        tc.tile_pool(name="sqp", bufs=2) as sqp,
        tc.tile_pool(name="rowp", bufs=2) as rowp,
        tc.tile_pool(name="outp", bufs=2) as outp,
        tc.tile_pool(name="psmm", bufs=4, space=bass.MemorySpace.PSUM) as psmm,
    ):
        # all-ones [128, 128] bf16 stationary operand: ones_mat.T @ sq gives
        # the per-column sums replicated across all 128 partitions, fusing
        # the partition-reduce and the broadcast into one full-speed matmul
        ones_f32 = pers.tile([KP, KP], F32, tag="ones_f32")
        nc.vector.memset(ones_f32, 1.0)
        ones_mat = pers.tile([KP, KP], BF16, tag="ones_mat")
        nc.scalar.copy(ones_mat, ones_f32)

        cur_fr = pers.tile([KP, NK, HW], BF16, tag="cur_fr")
        ref_n = [
            pers.tile([KP, NK, NT], BF16, tag=f"ref_n{n}", name=f"ref_n{n}")
            for n in range(NN)
        ]

        cur_r = cur.rearrange("(k p) n -> p k n", p=KP)
        ref_r = ref.rearrange("(k p) n -> p k n", p=KP)
        sim_pm = sim.rearrange("(mm p) n -> p mm n", p=KP)

        def body():
            # fill: ref slices on the HWDGE/sync queue, cur slices on
            # SWDGE/gpsimd in parallel (the output DMAs start later)
            for n in range(NN):
                sl = slice(n * NT, (n + 1) * NT)
                nc.sync.dma_start(out=ref_n[n], in_=ref_r[:, :, sl])
                nc.gpsimd.dma_start(out=cur_fr[:, :, sl], in_=cur_r[:, :, sl])

            def norm_chain(dst_ap, n, which):
                """L2-normalize 512 pixels x 256 channels of dst in place."""
                sq = sqp.tile([KP, NK, NT], BF16, tag=f"sq_{which}", name="sq")
                nc.vector.tensor_mul(sq, dst_ap, dst_ap)  # DVE 2x packed bf16
                # norm-phase PSUM lives in half of a psmm pair tile so the
                # main loop can use all 8 banks
                pb = psmm.tile([KP, 2 * NT], F32, tag="mm", name="pb")[:, :NT]
                for k in range(NK):
                    nc.tensor.matmul(
                        pb, ones_mat, sq[:, k, :], start=(k == 0), stop=(k == NK - 1)
                    )
                # 1/sqrt as reciprocal (DVE, from PSUM) then Sqrt (ACT -> bf16);
                # the Rsqrt ACT table is framework-blocked for accuracy
                rec = rowp.tile([KP, NT], F32, tag="rec", name="rec")
                nc.vector.reciprocal(rec, pb)
                inv = rowp.tile([KP, NT], BF16, tag="inv", name="inv")
                nc.scalar.activation(inv, rec, ACTF.Sqrt)
                for k in range(NK):
                    nc.vector.tensor_mul(dst_ap[:, k, :], dst_ap[:, k, :], inv)

            # every chain starts as soon as its input slice lands; ref and
            # cur chains interleave so neither queue's data waits
            for n in range(NN):
                norm_chain(ref_n[n], n, "r")
                norm_chain(cur_fr[:, :, n * NT:(n + 1) * NT], n, "c")

            def out_group(mo, msz, gi):
                out_sb = outp.tile([KP, MO, HW], BF16, tag="out", name="out_sb")
                for mi in range(msz):
                    m = mo + mi
                    for p in range(NP):
                        ps2 = psmm.tile([KP, 2 * NT], F32, tag="mm", name="ps2")
                        for ni in range(2):
                            n = 2 * p + ni
                            for k in range(NK):
                                nc.tensor.matmul(
                                    ps2[:, ni * NT:(ni + 1) * NT],
                                    cur_fr[:, k, m * MT:(m + 1) * MT],
                                    ref_n[n][:, k, :],
                                    start=(k == 0),
                                    stop=(k == NK - 1),
                                )
                        dst = out_sb[:, mi, 2 * p * NT:2 * (p + 1) * NT]
                        # 5:3 ACT:DVE split balances total engine load (DVE
                        # also carries the squares/recips/muls of the norms)
                        if (m * NP + p) % 8 < 5:
                            nc.scalar.copy(dst, ps2)
                        else:
                            nc.vector.tensor_copy(dst, ps2)
                # alternate the issuing queue (SP HWDGE / Pool SWDGE) so each
                # DMA's descriptor-gen overhead hides under the other's
                # in-flight transfer
                eng = nc.sync if gi % 2 == 0 else nc.gpsimd
                eng.dma_start(out=sim_pm[:, mo:mo + msz, :], in_=out_sb[:, :msz, :])

            groups = [1, 2, 3, 3] + [3] * 7 + [2]
            mo = 0
            for gi, msz in enumerate(groups):
                out_group(mo, msz, gi)
                mo += msz
            assert mo == NM

        if loop_iters == 1:
            body()
        else:
            hints = (
                mybir.EngineType.PE,
                mybir.EngineType.Activation,
                mybir.EngineType.DVE,
            )
            with tc.For_i(0, loop_iters, 1, hint_engines=hints):
                body()


_NC_CACHE = {}


def _np_bf16(x):
    return np.asarray(x).astype(mybir.dt.np(BF16))


def _timing_input_arrays(cur, ref):
    """Map device-input tensor names -> per-batch host arrays for test.py."""
    return {"cur": _np_bf16(cur), "ref": _np_bf16(ref)}


def _get_nc(loop_iters=1, cfg=None):
    key = ("nc", loop_iters)
    if key not in _NC_CACHE:
        nc = bacc.Bacc("TRN2", target_bir_lowering=False, debug=False)
        cur_d = nc.dram_tensor("cur", [C, HW], BF16, kind="ExternalInput")
        ref_d = nc.dram_tensor("ref", [C, HW], BF16, kind="ExternalInput")
        sim_d = nc.dram_tensor("sim", [HW, HW], BF16, kind="ExternalOutput")
        with tile.TileContext(nc) as tc:
            _kernel_body(tc, cur_d.ap(), ref_d.ap(), sim_d.ap(), loop_iters=loop_iters)
        nc.compile()
        _NC_CACHE[key] = nc
    return _NC_CACHE[key]


def kernel(ref_features, cur_features, _run_kwargs=None):
    ref_np = _np_bf16(
        np.ascontiguousarray(np.asarray(ref_features, dtype=np.float32)).reshape(
            B, C, HW
        )
    )
    cur_np = _np_bf16(
        np.ascontiguousarray(np.asarray(cur_features, dtype=np.float32)).reshape(
            B, C, HW
        )
    )
    nc = _get_nc()
    in_maps = [{"cur": cur_np[b], "ref": ref_np[b]} for b in range(B)]
    res = bass_utils.run_bass_kernel_spmd(
        nc, in_maps, core_ids=list(range(B)), **(_run_kwargs or {})
    )
    out = np.stack(
        [np.asarray(res.results[b]["sim"]).astype(np.float32) for b in range(B)],
        axis=0,
    )
    if _run_kwargs is not None:
        _NC_CACHE["last_results"] = res
    return out.reshape(B, H, W, H, W)


# revision 15
# speedup vs baseline: 1.0457x; 1.0457x over previous
"""Trainium2 Bass kernel for CorrCosine cost volumes.

Inputs (full): ref_features, cur_features [8, 256, 64, 64] f32.
out[b, hc, wc, hr, wr] = <cur_n[b, :, hc, wc], ref_n[b, :, hr, wr]>
where *_n are channel-L2-normalized features.

Sharding: data-parallel over batch B=8 across the 8 NeuronCores — each core
computes one batch's full [4096, 4096] cosine-similarity matrix:
  sim = (cur / |cur|_C).T @ (ref / |ref|_C)   with K = C = 256.

Per-core structure (Tile framework), bf16 pipeline (measured rates from
For_i microbenchmarks on this HW: PE 512 matmuls 113 us, 36 MB DMA 108 us
(~333 GB/s), 128 paired PSUM->SBUF copies 77 us):
  - host converts both inputs to bf16 (4 MB in); output written bf16
    (32 MB out, upcast to f32 on the host). bf16 matmuls run 1 cycle/row
    at 2.4 GHz unconditionally.
  - both inputs live as 8 separate 512-pixel tiles so Tile's dependency
    tracking stays fine-grained: the first main matmul only waits for
    ref tiles 0-1 + cur tile 0, not for the whole 4 MB fill. ref slices
    split across BOTH DMA queues (sync HWDGE + gpsimd SWDGE) so all of
    ref lands in ~5 us; every output column needs all of ref first.
  - ref chains: square -> all-ones [128,128] bf16 matmul per K-chunk
    (partition-reduce AND broadcast in one full-speed op) -> sqrt+recip
    -> in-place normalize (bf16 DVE 2x-packed muls).
  - cur stays unnormalized; inverse norms are reduced into output-row
    layout [128, 32] via N=1 matmuls and folded into the mandatory
    PSUM->SBUF copies for free. cur chains 2-7 are emitted after output
    group 2: PE executes in order, so putting them early would block PE
    on the last cur slices while main matmuls are already runnable.
  - main loop: per m-chunk 4 PSUM pair-tiles [128, 1024] (2 banks each,
    psmm bufs=4 = all 8 banks so PE never waits on copy drain), 4 bf16
    matmuls per pair, paired scaled copies (fold inv_cur, cast f32->bf16)
    alternating ScalarE/VectorE, 3 MB output DMAs alternating the
    sync/gpsimd queues (measured best).
  - rel-err ~3e-3 overall vs the 2e-2 gate.

loop_iters>1 wraps the body in a tc.For_i hardware loop for timing: device
exec then spans hundreds of ms, which is measurable through the axon tunnel
(single-shot exec hides entirely under the ~90 ms dispatch latency).
"""

import numpy as np

import concourse.bass as bass
import concourse.mybir as mybir
import concourse.tile as tile
from concourse import bacc, bass_utils

B, C, H, W = 8, 256, 64, 64
HW = H * W           # 4096 pixels
KP = 128             # partitions per K-chunk
NK = C // KP         # 2 K-chunks
MT = 128             # output partition tile (cur pixels)
NT = 512             # output free tile (ref pixels) = one f32 PSUM bank
NM = HW // MT        # 32 m-chunks
NN = HW // NT        # 8 n-tiles
NP = NN // 2         # 4 n-pairs per m-chunk
MO = 3               # m-chunks per output DMA (3 MB bf16 per dma_start)
MPS = NT // MT       # m-chunks per 512-pixel input slice = 4

F32 = mybir.dt.float32
BF16 = mybir.dt.bfloat16
ACTF = mybir.ActivationFunctionType


def _kernel_body(tc, cur, ref, sim, loop_iters=1):
    nc = tc.nc
    with (
        nc.allow_low_precision(
            reason="bf16 norm math; 2e-2 rel-err gate, measured ~3e-3"
        ),
        tc.tile_pool(name="pers", bufs=1) as pers,
        tc.tile_pool(name="sqp", bufs=2) as sqp,
        tc.tile_pool(name="rowp", bufs=2) as rowp,
        tc.tile_pool(name="outp", bufs=2) as outp,
        tc.tile_pool(name="psmm", bufs=4, space=bass.MemorySpace.PSUM) as psmm,
    ):
        ones_col_f = pers.tile([KP, 1], F32, tag="ones_col_f")
        nc.vector.memset(ones_col_f, 1.0)
        ones_col = pers.tile([KP, 1], BF16, tag="ones_col")
        nc.scalar.copy(ones_col, ones_col_f)
        # all-ones [128, 128] bf16 stationary operand: ones_mat.T @ sq gives
        # the per-column sums replicated across all 128 partitions, fusing
        # the partition-reduce and the broadcast into one full-speed matmul
        ones_f32 = pers.tile([KP, KP], F32, tag="ones_f32")
        nc.vector.memset(ones_f32, 1.0)
        ones_mat = pers.tile([KP, KP], BF16, tag="ones_mat")
        nc.scalar.copy(ones_mat, ones_f32)

        cur_sl = [
            pers.tile([KP, NK, NT], BF16, tag=f"cur{n}", name=f"cur{n}")
            for n in range(NN)
        ]
        ref_n = [
            pers.tile([KP, NK, NT], BF16, tag=f"ref_n{n}", name=f"ref_n{n}")
            for n in range(NN)
        ]
        inv_cur = pers.tile([KP, NM], F32, tag="inv_cur")

        cur_r = cur.rearrange("(k p) n -> p k n", p=KP)
        ref_r = ref.rearrange("(k p) n -> p k n", p=KP)
        sim_pm = sim.rearrange("(mm p) n -> p mm n", p=KP)

        def body():
            # ref first, split across both queues, so all 8 ref tiles land
            # in ~5 us (every output column needs all of ref); cur follows
            for n in range(NN):
                sl = slice(n * NT, (n + 1) * NT)
                eng = nc.sync if n % 2 == 0 else nc.gpsimd
                eng.dma_start(out=ref_n[n], in_=ref_r[:, :, sl])
            for n in range(NN):
                sl = slice(n * NT, (n + 1) * NT)
                eng = nc.sync if n % 2 == 0 else nc.gpsimd
                eng.dma_start(out=cur_sl[n], in_=cur_r[:, :, sl])

            def ref_chain(n):
                sq = sqp.tile([KP, NK, NT], BF16, tag="sq_r", name="sq_r")
                nc.vector.tensor_mul(sq, ref_n[n], ref_n[n])  # DVE 2x packed
                # norm-phase PSUM lives in half of a psmm pair tile so the
                # main loop can use all 8 banks
                pb = psmm.tile([KP, 2 * NT], F32, tag="mm", name="pb")[:, :NT]
                for k in range(NK):
                    nc.tensor.matmul(
                        pb, ones_mat, sq[:, k, :], start=(k == 0), stop=(k == NK - 1)
                    )
                srt = rowp.tile([KP, NT], F32, tag="srt", name="srt")
                nc.scalar.activation(srt, pb, ACTF.Sqrt)
                inv = rowp.tile([KP, NT], BF16, tag="inv", name="inv")
                nc.vector.reciprocal(inv, srt)
                for k in range(NK):
                    nc.vector.tensor_mul(ref_n[n][:, k, :], ref_n[n][:, k, :], inv)

            def cur_chain(n):
                csq = sqp.tile([KP, NK, NT], BF16, tag="sq_c", name="sq_c")
                if n % 2 == 0:
                    nc.vector.tensor_mul(csq, cur_sl[n], cur_sl[n])
                else:
                    nc.scalar.activation(csq, cur_sl[n], ACTF.Square)
                pcol = psmm.tile([KP, 2 * NT], F32, tag="mm", name="pcol")[:, :MPS]
                for j in range(MPS):
                    for k in range(NK):
                        nc.tensor.matmul(
                            pcol[:, j:j + 1],
                            csq[:, k, j * MT:(j + 1) * MT],
                            ones_col,
                            start=(k == 0),
                            stop=(k == NK - 1),
                        )
                ncur = rowp.tile([KP, MPS], F32, tag="ncur", name="ncur")
                nc.scalar.activation(ncur, pcol, ACTF.Sqrt)
                nc.vector.reciprocal(inv_cur[:, n * MPS:(n + 1) * MPS], ncur)

            # ref chains first (every output column needs all of ref);
            # cur chains 0-1 cover the first 8 m-chunks, the rest are
            # deferred into the main loop so in-order PE never blocks on a
            # not-yet-loaded cur slice
            for n in range(NN):
                ref_chain(n)
                if n < 2:
                    cur_chain(n)

            def out_group(mo, msz, gi):
                out_sb = outp.tile([KP, MO, HW], BF16, tag="out", name="out_sb")
                for mi in range(msz):
                    m = mo + mi
                    for p in range(NP):
                        ps2 = psmm.tile([KP, 2 * NT], F32, tag="mm", name="ps2")
                        for ni in range(2):
                            n = 2 * p + ni
                            for k in range(NK):
                                nc.tensor.matmul(
                                    ps2[:, ni * NT:(ni + 1) * NT],
                                    cur_sl[m // MPS][:, k, (m % MPS) * MT:(m % MPS + 1) * MT],
                                    ref_n[n][:, k, :],
                                    start=(k == 0),
                                    stop=(k == NK - 1),
                                )
                        dst = out_sb[:, mi, 2 * p * NT:2 * (p + 1) * NT]
                        if p % 2 == 0:
                            nc.scalar.mul(dst, ps2, inv_cur[:, m:m + 1])
                        else:
                            nc.vector.tensor_scalar_mul(dst, ps2, inv_cur[:, m:m + 1])
                # alternate the issuing queue (SP HWDGE / Pool SWDGE) so each
                # DMA's descriptor-gen overhead hides under the other's
                # in-flight transfer
                eng = nc.sync if gi % 2 == 0 else nc.gpsimd
                eng.dma_start(out=sim_pm[:, mo:mo + msz, :], in_=out_sb[:, :msz, :])

            groups = [1, 2, 3, 3] + [3] * 7 + [2]
            mo = 0
            for gi, msz in enumerate(groups):
                out_group(mo, msz, gi)
                mo += msz
                if gi == 2:
                    for n in range(2, NN):
                        cur_chain(n)
            assert mo == NM

        if loop_iters == 1:
            body()
        else:
            hints = (
                mybir.EngineType.PE,
                mybir.EngineType.Activation,
                mybir.EngineType.DVE,
            )
            with tc.For_i(0, loop_iters, 1, hint_engines=hints):
                body()


_NC_CACHE = {}


def _np_bf16(x):
    return np.asarray(x).astype(mybir.dt.np(BF16))


def _timing_input_arrays(cur, ref):
    """Map device-input tensor names -> per-batch host arrays for test.py."""
    return {"cur": _np_bf16(cur), "ref": _np_bf16(ref)}


def _get_nc(loop_iters=1, cfg=None):
    key = ("nc", loop_iters)
    if key not in _NC_CACHE:
        nc = bacc.Bacc("TRN2", target_bir_lowering=False, debug=False)
        cur_d = nc.dram_tensor("cur", [C, HW], BF16, kind="ExternalInput")
        ref_d = nc.dram_tensor("ref", [C, HW], BF16, kind="ExternalInput")
        sim_d = nc.dram_tensor("sim", [HW, HW], BF16, kind="ExternalOutput")
        with tile.TileContext(nc) as tc:
            _kernel_body(tc, cur_d.ap(), ref_d.ap(), sim_d.ap(), loop_iters=loop_iters)
        nc.compile()
        _NC_CACHE[key] = nc
    return _NC_CACHE[key]


def kernel(ref_features, cur_features, _run_kwargs=None):
    ref_np = _np_bf16(
        np.ascontiguousarray(np.asarray(ref_features, dtype=np.float32)).reshape(
            B, C, HW
        )
    )
    cur_np = _np_bf16(
        np.ascontiguousarray(np.asarray(cur_features, dtype=np.float32)).reshape(
            B, C, HW
        )
    )
    nc = _get_nc()
    in_maps = [{"cur": cur_np[b], "ref": ref_np[b]} for b in range(B)]
    res = bass_utils.run_bass_kernel_spmd(
        nc, in_maps, core_ids=list(range(B)), **(_run_kwargs or {})
    )
    out = np.stack(
        [np.asarray(res.results[b]["sim"]).astype(np.float32) for b in range(B)],
        axis=0,
    )
    if _run_kwargs is not None:
        _NC_CACHE["last_results"] = res
    return out.reshape(B, H, W, H, W)
